# revision 25
# baseline (speedup 1.0000x reference)
"""Trainium2 Bass kernel for nn_AttentionBlock (B=8, C=128, H=W=64).

Data-parallel over batch across 8 NeuronCores (one batch element per core).
Per core, full 4096x4096 single-head attention:

  xt = x.T;  q = xt@(qw*scale) + qb*scale;  k = xt@kw;  v0 = xt@vw
  S = q k^T;  P = softmax(S);  out = xt + P@v0@pw + (vb@pw + pb)
  (k-bias is softmax-invariant; v-bias folds through rowsum==1)

Baseline dataflow + software-pipelined emission: the Tile scheduler pops
ready work by emission priority, so the S+softmax phase of group g is
EMITTED interleaved with the transpose/PV phase of group g-1 — PE
streams transposes/PV while DVE does maxes and ScalarE does exps of the
next group concurrently. Everything else (dtypes, DMAs, pools, drain
split) matches the baseline exactly; p_pool/ptc go 6->8 so two groups
of P tiles can be live at once.
"""

import numpy as np

C = 128
N = 4096  # tokens per batch element (64*64)
B = 8
H = W = 64

MMW = 512           # S matmul moving width (fp16 moving operand cap)
NDEF = 2            # chunks exp'd with the true row max (deferred)

_cache = {}


def _build(n_tokens=N):
    import concourse.bass as bass
    import concourse.mybir as mybir
    import concourse.tile as tile
    from concourse import bacc
    from concourse.masks import make_identity

    f32 = mybir.dt.float32
    f16 = mybir.dt.float16
    Alu = mybir.AluOpType
    Act = mybir.ActivationFunctionType

    NTOK = n_tokens
    NTILES = NTOK // 128          # q-token row tiles
    MCHUNK = min(1024, NTOK)      # S psum chunk width (2 banks)
    MCH_CNT = NTOK // MCHUNK      # chunks per row-tile
    GRP = min(512, NTOK)          # PV n-group size
    TPG = GRP // 128              # row-tiles per group
    NGRP = NTOK // GRP
    MBLK = NTOK // 128            # m blocks total
    NCOR = MCH_CNT - NDEF

    nc = bacc.Bacc("TRN2", target_bir_lowering=False, debug=False, num_devices=8)

    xh_e = nc.dram_tensor("xh", [C, NTOK], f16, kind="ExternalInput")
    xpb_e = nc.dram_tensor("xpb", [NTOK, C], f32, kind="ExternalInput")
    qw_e = nc.dram_tensor("qw", [C, C], f16, kind="ExternalInput")
    kw_e = nc.dram_tensor("kw", [C, C], f16, kind="ExternalInput")
    vw_e = nc.dram_tensor("vw", [C, C], f16, kind="ExternalInput")
    pw_e = nc.dram_tensor("pw", [C, C], f16, kind="ExternalInput")
    qb_e = nc.dram_tensor("qb", [C, 1], f32, kind="ExternalInput")
    out_e = nc.dram_tensor("out", [NTOK, C], f32, kind="ExternalOutput")

    with tile.TileContext(nc) as tc:
        with (
            tc.tile_pool(name="persist", bufs=1) as persist,
            tc.tile_pool(name="pp", bufs=6) as p_pool,
            tc.tile_pool(name="ptc", bufs=8) as ptc_pool,
            tc.tile_pool(name="small", bufs=2) as small,
            tc.tile_pool(name="stats", bufs=6) as stats,
            tc.tile_pool(name="ivp", bufs=2 * TPG + 2) as ivp,
            tc.tile_pool(name="psA", bufs=1, space="PSUM") as psA,
            tc.tile_pool(name="psS", bufs=3, space="PSUM") as psS,
            tc.tile_pool(name="psT", bufs=1, space="PSUM") as psT,
        ):
            # ---- constants / weights ----
            qw_sb = persist.tile([C, C], f16, tag="qw")
            kw_sb = persist.tile([C, C], f16, tag="kw")
            vw_sb = persist.tile([C, C], f16, tag="vw")
            pw_sb = persist.tile([C, C], f16, tag="pw")
            qb_sb = persist.tile([C, 1], f32, tag="qb")
            # xh (1MB, gates everything) goes FIRST on the SWDGE queue;
            # the small weight loads ride HWDGE (sync) in parallel
            xh_sb = persist.tile([C, NTOK], f16, tag="xh")
            nc.gpsimd.dma_start(out=xh_sb[:], in_=xh_e[:])
            nc.sync.dma_start(out=qw_sb[:], in_=qw_e[:])
            nc.sync.dma_start(out=kw_sb[:], in_=kw_e[:])
            nc.sync.dma_start(out=vw_sb[:], in_=vw_e[:])
            nc.sync.dma_start(out=qb_sb[:], in_=qb_e[:])
            nc.sync.dma_start(out=pw_sb[:], in_=pw_e[:])

            ident = persist.tile([C, C], f16, tag="ident")
            make_identity(nc, ident[:])

            # ---- QT / KT / VT (c_out, n) fp16 ----
            QT = persist.tile([C, NTOK], f16, tag="QT")
            KT = persist.tile([C, NTOK], f16, tag="KT")
            VT = persist.tile([C, NTOK], f16, tag="VT")
            for j in range(NTOK // 512):
                sl = slice(j * 512, (j + 1) * 512)
                pq = psS.tile([C, 1024], f32, tag="s", name="pq")
                nc.tensor.matmul(pq[:, :512], lhsT=qw_sb[:], rhs=xh_sb[:, sl])
                nc.vector.tensor_scalar(
                    out=QT[:, sl], in0=pq[:, :512], scalar1=qb_sb[:],
                    scalar2=None, op0=Alu.add,
                )
                pk = psS.tile([C, 1024], f32, tag="s", name="pk")
                nc.tensor.matmul(pk[:, :512], lhsT=kw_sb[:], rhs=xh_sb[:, sl])
                nc.scalar.activation(out=KT[:, sl], in_=pk[:, :512],
                                     func=Act.Copy)
                pvt = psS.tile([C, 1024], f32, tag="s", name="pvt")
                nc.tensor.matmul(pvt[:, :512], lhsT=vw_sb[:],
                                 rhs=xh_sb[:, sl])
                nc.scalar.activation(out=VT[:, sl], in_=pvt[:, :512],
                                     func=Act.Copy)

            # ---- V in (m, c) layout: V[i*128+p, c] at V_sb[p, i, c] ----
            # PE-transpose VT 128-blocks through psT, 8 per drain; emitted
            # as units interleaved with group 0's S window (ramp)
            V_sb = persist.tile([C, MBLK, 128], f16, tag="V")

            def v_units():
                def mk_v(r):
                    def emit():
                        tp = psT.tile([C, 8 * 128], f16, tag="t", name="tpv")
                        for k in range(8):
                            i = 8 * r + k
                            nc.tensor.transpose(
                                tp[:, k * 128:(k + 1) * 128],
                                VT[:, i * 128:(i + 1) * 128], ident[:],
                            )
                        if r % 2 == 0:
                            nc.vector.tensor_copy(
                                V_sb[:, 8 * r:8 * (r + 1), :], tp[:])
                        else:
                            nc.scalar.activation(
                                out=V_sb[:, 8 * r:8 * (r + 1), :], in_=tp[:],
                                func=Act.Copy)
                    return emit
                return [mk_v(r) for r in range(MBLK // 8)]

            iv_tiles = [None] * NTILES
            P_groups = [None] * NGRP   # per-group list of P_t tiles
            ptc_tiles = [[None] * (MBLK // 8) for _ in range(NGRP)]
            late_fills = [None] * NGRP  # last tile's fill units, deferred

            def s_units_for_group(g):
                """Emission closures for group g's S+softmax (per tile:
                MCH_CNT chunk units + 1 stats unit)."""
                units = []
                P_tiles = []
                P_groups[g] = P_tiles
                for t in range(TPG):
                    nt = g * TPG + t
                    qsl = slice(nt * 128, (nt + 1) * 128)
                    state = {}

                    def mk_chunk(h, nt=nt, qsl=qsl, state=state,
                                 P_tiles=P_tiles):
                        def emit():
                            if h == 0:
                                state["nm"] = stats.tile(
                                    [C, MCH_CNT], f32, tag="nm", name="nm")
                                state["rsc"] = stats.tile(
                                    [C, MCH_CNT], f32, tag="rsc", name="rsc")
                                state["P_t"] = p_pool.tile(
                                    [C, NTOK], f16, tag="P", name="P_t")
                                state["defer"] = []
                                P_tiles.append(state["P_t"])
                            nm, rsc, P_t = (state["nm"], state["rsc"],
                                            state["P_t"])
                            sps = psS.tile([C, MCHUNK], f32, tag="s",
                                           name="sps")
                            for q in range(MCHUNK // MMW):
                                nc.tensor.matmul(
                                    sps[:, q * MMW:(q + 1) * MMW],
                                    lhsT=QT[:, qsl],
                                    rhs=KT[:, h * MCHUNK + q * MMW:
                                           h * MCHUNK + (q + 1) * MMW],
                                )
                            nc.vector.tensor_reduce(
                                out=nm[:, h:h + 1], in_=sps[:],
                                axis=mybir.AxisListType.X, op=Alu.max,
                                negate=True,
                            )
                            if h < NCOR:
                                nc.scalar.activation(
                                    out=P_t[:, h * MCHUNK:(h + 1) * MCHUNK],
                                    in_=sps[:], func=Act.Exp,
                                    bias=nm[:, h:h + 1], scale=1.0,
                                    accum_out=rsc[:, h:h + 1],
                                )
                            else:
                                state["defer"].append(sps)
                        return emit

                    def mk_stats(nt=nt, state=state):
                        def emit():
                            nm, rsc, P_t = (state["nm"], state["rsc"],
                                            state["P_t"])
                            ngm = stats.tile([C, 1], f32, tag="ngm",
                                             name="ngm")
                            nc.vector.tensor_reduce(
                                out=ngm[:], in_=nm[:],
                                axis=mybir.AxisListType.X, op=Alu.min,
                            )
                            for k, sps in enumerate(state["defer"]):
                                h = NCOR + k
                                nc.scalar.activation(
                                    out=P_t[:, h * MCHUNK:(h + 1) * MCHUNK],
                                    in_=sps[:], func=Act.Exp,
                                    bias=ngm[:], scale=1.0,
                                    accum_out=rsc[:, h:h + 1],
                                )
                            if NCOR > 0:
                                corrs = stats.tile([C, NCOR], f32,
                                                   tag="corrs", name="corrs")
                                nc.scalar.activation(
                                    out=corrs[:], in_=nm[:, 0:NCOR],
                                    func=Act.Exp, bias=ngm[:], scale=-1.0,
                                )
                                nc.vector.tensor_tensor(
                                    out=rsc[:, 0:NCOR], in0=rsc[:, 0:NCOR],
                                    in1=corrs[:], op=Alu.mult,
                                )
                                for h in range(NCOR):
                                    nc.vector.tensor_scalar(
                                        out=P_t[:, h * MCHUNK:
                                                (h + 1) * MCHUNK],
                                        in0=P_t[:, h * MCHUNK:
                                                (h + 1) * MCHUNK],
                                        scalar1=corrs[:, h:h + 1],
                                        scalar2=None, op0=Alu.mult,
                                    )
                            rs = stats.tile([C, 1], f32, tag="rs", name="rs")
                            nc.vector.tensor_reduce(
                                out=rs[:], in_=rsc[:],
                                axis=mybir.AxisListType.X, op=Alu.add,
                            )
                            iv = ivp.tile([C, 1], f32, tag="iv", name="iv")
                            nc.vector.reciprocal(iv[:], rs[:])
                            iv_tiles[nt] = iv
                        return emit

                    def mk_fill(r, t=t, state=state):
                        # transpose 8 m-blocks of THIS tile right after its
                        # softmax -> keeps PE warm inside the S window
                        def emit():
                            if t == 0:
                                ptc_tiles[g][r] = ptc_pool.tile(
                                    [C, TPG, 1024], f16, tag="pt",
                                    name="ptc")
                            ptc_r = ptc_tiles[g][r]
                            P_t = state["P_t"]
                            tp = psT.tile([C, 1024], f16, tag="t", name="tp")
                            for k in range(8):
                                i = 8 * r + k
                                nc.tensor.transpose(
                                    tp[:, k * 128:(k + 1) * 128],
                                    P_t[:, i * 128:(i + 1) * 128],
                                    ident[:],
                                )
                            d = 4 * r + t
                            if d % 2 == 0 or d == 5:
                                nc.vector.tensor_copy(ptc_r[:, t, :], tp[:])
                            else:
                                nc.scalar.activation(out=ptc_r[:, t, :],
                                                     in_=tp[:],
                                                     func=Act.Copy)
                        return emit

                    for h in range(MCH_CNT):
                        units.append(mk_chunk(h))
                    units.append(mk_stats())
                    if t < TPG - 1:
                        for r in range(MBLK // 8):
                            units.append(mk_fill(r))
                    else:
                        late_fills[g] = [mk_fill(r)
                                         for r in range(MBLK // 8)]
                return units

            def tpv_units_for_group(g):
                """Emission closures for group g's transpose/PV/epilogue."""
                units = []
                state = {}

                def mk_pv(r, state=state):
                    def emit():
                        if r == 0:
                            state["O_ps"] = psA.tile([C, GRP], f32, tag="a",
                                                     name="O_ps")
                            xpb_g = small.tile([C, TPG, 128], f32, tag="xpb",
                                               name="xpb_g")
                            state["xpb_g"] = xpb_g
                            nc.gpsimd.dma_start(
                                out=xpb_g[:],
                                in_=xpb_e[g * GRP:(g + 1) * GRP, :].rearrange(
                                    "(t p) c -> p t c", p=128),
                            )
                        O_ps = state["O_ps"]
                        ptc_r = ptc_tiles[g][r]
                        for k in range(8):
                            i = 8 * r + k
                            nc.tensor.matmul(
                                O_ps[:], lhsT=V_sb[:, i, :],
                                rhs=ptc_r[:, :, k * 128:(k + 1) * 128],
                                start=(i == 0), stop=(i == MBLK - 1),
                            )
                    return emit

                def mk_epi(state=state):
                    def emit():
                        O_ps = state["O_ps"]
                        O_sb = small.tile([C, GRP], f16, tag="O", name="O_sb")
                        nc.scalar.activation(out=O_sb[:], in_=O_ps[:],
                                             func=Act.Copy)
                        H_ps = psA.tile([C, GRP], f32, tag="a", name="H_ps")
                        nc.tensor.matmul(H_ps[:], lhsT=pw_sb[:], rhs=O_sb[:])
                        Hs = small.tile([C, GRP], f16, tag="Hs", name="Hs")
                        nc.scalar.activation(out=Hs[:], in_=H_ps[:],
                                             func=Act.Copy)
                        tph = psA.tile([C, GRP], f16, tag="a", name="tph")
                        for t in range(TPG):
                            nc.tensor.transpose(
                                tph[:, t * 128:(t + 1) * 128],
                                Hs[:, t * 128:(t + 1) * 128], ident[:],
                            )
                        xpb_g = state["xpb_g"]
                        out_g = small.tile([C, TPG, 128], f32, tag="og",
                                           name="out_g")
                        for t in range(TPG):
                            nt = g * TPG + t
                            nc.vector.scalar_tensor_tensor(
                                out=out_g[:, t, :],
                                in0=tph[:, t * 128:(t + 1) * 128],
                                scalar=iv_tiles[nt][:], in1=xpb_g[:, t, :],
                                op0=Alu.mult, op1=Alu.add,
                            )
                        nc.gpsimd.dma_start(
                            out=out_e[g * GRP:(g + 1) * GRP, :].rearrange(
                                "(t p) c -> p t c", p=128),
                            in_=out_g[:],
                        )
                    return emit

                for r in range(MBLK // 8):
                    units.append(late_fills[g][r])
                    units.append(mk_pv(r))
                units.append(mk_epi())
                return units

            # ---- interleaved emission: S(g) with TPV(g-1) ----
            for g in range(NGRP + 1):
                su = s_units_for_group(g) if g < NGRP else []
                tu = tpv_units_for_group(g - 1) if g > 0 else v_units()
                ns, ntp = len(su), len(tu)
                total = max(ns, ntp)
                si = ti = 0
                for k in range(total):
                    while ti * total <= k * ntp and ti < ntp:
                        tu[ti]()
                        ti += 1
                    while si * total <= k * ns and si < ns:
                        su[si]()
                        si += 1
                while ti < ntp:
                    tu[ti]()
                    ti += 1
                while si < ns:
                    su[si]()
                    si += 1

    nc.compile()
    return nc


def _get_nc(n_tokens=N):
    if n_tokens not in _cache:
        _cache[n_tokens] = _build(n_tokens)
    return _cache[n_tokens]


def prep_inputs(x, qw, qb, kw, kb, vw, vb, proj_w, proj_b, n_tokens=N):
    """Host-side prep: shard over batch, fold scale/biases, transpose."""
    x = np.asarray(x, dtype=np.float32)
    b, c, h, w = x.shape
    scale = c ** (-0.5)
    qw_s = (np.asarray(qw, np.float32) * scale).astype(np.float16)
    kw16 = np.asarray(kw, np.float32).astype(np.float16)
    vw16 = np.asarray(vw, np.float32).astype(np.float16)
    pw16 = np.asarray(proj_w, np.float32).astype(np.float16)
    qb_s = (np.asarray(qb, np.float32) * scale).reshape(c, 1).astype(np.float32)
    pb2 = (np.asarray(vb, np.float32) @ np.asarray(proj_w, np.float32)
           + np.asarray(proj_b, np.float32)).astype(np.float32)

    in_maps = []
    for i in range(b):
        xc = x[i].reshape(c, h * w)[:, :n_tokens]
        xt = xc.T.copy()
        in_maps.append({
            "xh": np.ascontiguousarray(xc).astype(np.float16),
            "xpb": np.ascontiguousarray(xt + pb2[None, :]),
            "qw": qw_s, "kw": kw16, "vw": vw16, "pw": pw16,
            "qb": qb_s,
        })
    return in_maps


def kernel(x, qw, qb, kw, kb, vw, vb, proj_w, proj_b, _trace=False):
    from concourse.bass_utils import run_bass_kernel_spmd

    nc = _get_nc(N)
    in_maps = prep_inputs(x, qw, qb, kw, kb, vw, vb, proj_w, proj_b)
    res = run_bass_kernel_spmd(nc, in_maps, core_ids=list(range(B)),
                               trace=_trace)
    kernel.last_results = res
    out = np.stack([np.asarray(res.results[i]["out"]) for i in range(B)])
    return out.reshape(B, H, W, C).astype(np.float32)


# revision 26
# speedup vs baseline: 1.2498x; 1.2498x over previous
"""Trainium2 Bass kernel for nn_AttentionBlock (B=8, C=128, H=W=64).

Data-parallel over batch across 8 NeuronCores (one batch element per core).
Per core, full 4096x4096 single-head attention:

  xt = x.T;  q = xt@(qw*scale) + qb*scale;  k = xt@kw;  v0 = xt@vw
  S = q k^T;  P = softmax(S);  out = xt + P@v0@pw + (vb@pw + pb)
  (k-bias is softmax-invariant; v-bias folds through rowsum==1)

Baseline dataflow + software-pipelined emission: the Tile scheduler pops
ready work by emission priority, so the S+softmax phase of group g is
EMITTED interleaved with the transpose/PV phase of group g-1 — PE
streams transposes/PV while DVE does maxes and ScalarE does exps of the
next group concurrently. Everything else (dtypes, DMAs, pools, drain
split) matches the baseline exactly; p_pool/ptc go 6->8 so two groups
of P tiles can be live at once.
"""

import numpy as np

C = 128
N = 4096  # tokens per batch element (64*64)
B = 8
H = W = 64

MMW = 512           # S matmul moving width (fp16 moving operand cap)
NDEF = 2            # chunks exp'd with the true row max (deferred)

_cache = {}


def _build(n_tokens=N):
    import concourse.bass as bass
    import concourse.mybir as mybir
    import concourse.tile as tile
    from concourse import bacc
    from concourse.masks import make_identity

    f32 = mybir.dt.float32
    f16 = mybir.dt.float16
    Alu = mybir.AluOpType
    Act = mybir.ActivationFunctionType

    NTOK = n_tokens
    NTILES = NTOK // 128          # q-token row tiles
    MCHUNK = min(1024, NTOK)      # S psum chunk width (2 banks)
    MCH_CNT = NTOK // MCHUNK      # chunks per row-tile
    GRP = min(512, NTOK)          # PV n-group size
    TPG = GRP // 128              # row-tiles per group
    NGRP = NTOK // GRP
    MBLK = NTOK // 128            # m blocks total
    NCOR = MCH_CNT - NDEF

    nc = bacc.Bacc("TRN2", target_bir_lowering=False, debug=False, num_devices=8)

    xh_e = nc.dram_tensor("xh", [C, NTOK], f16, kind="ExternalInput")
    xpb_e = nc.dram_tensor("xpb", [NTOK, C], f32, kind="ExternalInput")
    qw_e = nc.dram_tensor("qw", [C, C], f16, kind="ExternalInput")
    kw_e = nc.dram_tensor("kw", [C, C], f16, kind="ExternalInput")
    vw_e = nc.dram_tensor("vw", [C, C], f16, kind="ExternalInput")
    pw_e = nc.dram_tensor("pw", [C, C], f16, kind="ExternalInput")
    qb_e = nc.dram_tensor("qb", [C, 1], f32, kind="ExternalInput")
    out_e = nc.dram_tensor("out", [NTOK, C], f32, kind="ExternalOutput")

    with tile.TileContext(nc) as tc:
        with (
            tc.tile_pool(name="persist", bufs=1) as persist,
            tc.tile_pool(name="pp", bufs=6) as p_pool,
            tc.tile_pool(name="ptc", bufs=8) as ptc_pool,
            tc.tile_pool(name="small", bufs=2) as small,
            tc.tile_pool(name="stats", bufs=6) as stats,
            tc.tile_pool(name="ivp", bufs=2 * TPG + 2) as ivp,
            tc.tile_pool(name="psA", bufs=1, space="PSUM") as psA,
            tc.tile_pool(name="psS", bufs=3, space="PSUM") as psS,
            tc.tile_pool(name="psT", bufs=1, space="PSUM") as psT,
        ):
            # ---- constants / weights ----
            qw_sb = persist.tile([C, C], f16, tag="qw")
            kw_sb = persist.tile([C, C], f16, tag="kw")
            vw_sb = persist.tile([C, C], f16, tag="vw")
            pw_sb = persist.tile([C, C], f16, tag="pw")
            qb_sb = persist.tile([C, 1], f32, tag="qb")
            # xh (1MB, gates everything) goes FIRST on the SWDGE queue;
            # the small weight loads ride HWDGE (sync) in parallel
            xh_sb = persist.tile([C, NTOK], f16, tag="xh")
            nc.gpsimd.dma_start(out=xh_sb[:], in_=xh_e[:])
            nc.sync.dma_start(out=qw_sb[:], in_=qw_e[:])
            nc.sync.dma_start(out=kw_sb[:], in_=kw_e[:])
            nc.sync.dma_start(out=vw_sb[:], in_=vw_e[:])
            nc.sync.dma_start(out=qb_sb[:], in_=qb_e[:])
            nc.sync.dma_start(out=pw_sb[:], in_=pw_e[:])

            ident = persist.tile([C, C], f16, tag="ident")
            make_identity(nc, ident[:])

            # ---- QT / KT / VT (c_out, n) fp16 ----
            QT = persist.tile([C, NTOK], f16, tag="QT")
            KT = persist.tile([C, NTOK], f16, tag="KT")
            VT = persist.tile([C, NTOK], f16, tag="VT")
            for j in range(NTOK // 512):
                sl = slice(j * 512, (j + 1) * 512)
                pq = psS.tile([C, 1024], f32, tag="s", name="pq")
                nc.tensor.matmul(pq[:, :512], lhsT=qw_sb[:], rhs=xh_sb[:, sl])
                nc.vector.tensor_scalar(
                    out=QT[:, sl], in0=pq[:, :512], scalar1=qb_sb[:],
                    scalar2=None, op0=Alu.add,
                )
                pk = psS.tile([C, 1024], f32, tag="s", name="pk")
                nc.tensor.matmul(pk[:, :512], lhsT=kw_sb[:], rhs=xh_sb[:, sl])
                nc.scalar.activation(out=KT[:, sl], in_=pk[:, :512],
                                     func=Act.Copy)
                pvt = psS.tile([C, 1024], f32, tag="s", name="pvt")
                nc.tensor.matmul(pvt[:, :512], lhsT=vw_sb[:],
                                 rhs=xh_sb[:, sl])
                nc.scalar.activation(out=VT[:, sl], in_=pvt[:, :512],
                                     func=Act.Copy)

            # ---- V in (m, c) layout: V[i*128+p, c] at V_sb[p, i, c] ----
            # PE-transpose VT 128-blocks through psT, 8 per drain; emitted
            # as units interleaved with group 0's S window (ramp)
            V_sb = persist.tile([C, MBLK, 128], f16, tag="V")

            def v_units():
                def mk_v(r):
                    def emit():
                        tp = psT.tile([C, 8 * 128], f16, tag="t", name="tpv")
                        for k in range(8):
                            i = 8 * r + k
                            nc.tensor.transpose(
                                tp[:, k * 128:(k + 1) * 128],
                                VT[:, i * 128:(i + 1) * 128], ident[:],
                            )
                        if r % 2 == 0:
                            nc.vector.tensor_copy(
                                V_sb[:, 8 * r:8 * (r + 1), :], tp[:])
                        else:
                            nc.scalar.activation(
                                out=V_sb[:, 8 * r:8 * (r + 1), :], in_=tp[:],
                                func=Act.Copy)
                    return emit
                return [mk_v(r) for r in range(MBLK // 8)]

            iv_tiles = [None] * NTILES
            P_groups = [None] * NGRP   # per-group list of P_t tiles
            ptc_tiles = [[None] * (MBLK // 8) for _ in range(NGRP)]

            def s_units_for_group(g):
                """Emission closures for group g's S+softmax (per tile:
                MCH_CNT chunk units + 1 stats unit)."""
                units = []
                P_tiles = []
                P_groups[g] = P_tiles
                for t in range(TPG):
                    nt = g * TPG + t
                    qsl = slice(nt * 128, (nt + 1) * 128)
                    state = {}

                    def mk_chunk(h, nt=nt, qsl=qsl, state=state,
                                 P_tiles=P_tiles):
                        def emit():
                            if h == 0:
                                state["nm"] = stats.tile(
                                    [C, MCH_CNT], f32, tag="nm", name="nm")
                                state["rsc"] = stats.tile(
                                    [C, MCH_CNT], f32, tag="rsc", name="rsc")
                                state["P_t"] = p_pool.tile(
                                    [C, NTOK], f16, tag="P", name="P_t")
                                state["defer"] = []
                                P_tiles.append(state["P_t"])
                            nm, rsc, P_t = (state["nm"], state["rsc"],
                                            state["P_t"])
                            sps = psS.tile([C, MCHUNK], f32, tag="s",
                                           name="sps")
                            for q in range(MCHUNK // MMW):
                                nc.tensor.matmul(
                                    sps[:, q * MMW:(q + 1) * MMW],
                                    lhsT=QT[:, qsl],
                                    rhs=KT[:, h * MCHUNK + q * MMW:
                                           h * MCHUNK + (q + 1) * MMW],
                                )
                            nc.vector.tensor_reduce(
                                out=nm[:, h:h + 1], in_=sps[:],
                                axis=mybir.AxisListType.X, op=Alu.max,
                                negate=True,
                            )
                            if h < NCOR:
                                nc.scalar.activation(
                                    out=P_t[:, h * MCHUNK:(h + 1) * MCHUNK],
                                    in_=sps[:], func=Act.Exp,
                                    bias=nm[:, h:h + 1], scale=1.0,
                                    accum_out=rsc[:, h:h + 1],
                                )
                            else:
                                state["defer"].append(sps)
                        return emit

                    def mk_stats(nt=nt, state=state):
                        def emit():
                            nm, rsc, P_t = (state["nm"], state["rsc"],
                                            state["P_t"])
                            ngm = stats.tile([C, 1], f32, tag="ngm",
                                             name="ngm")
                            nc.vector.tensor_reduce(
                                out=ngm[:], in_=nm[:],
                                axis=mybir.AxisListType.X, op=Alu.min,
                            )
                            for k, sps in enumerate(state["defer"]):
                                h = NCOR + k
                                nc.scalar.activation(
                                    out=P_t[:, h * MCHUNK:(h + 1) * MCHUNK],
                                    in_=sps[:], func=Act.Exp,
                                    bias=ngm[:], scale=1.0,
                                    accum_out=rsc[:, h:h + 1],
                                )
                            if NCOR > 0:
                                corrs = stats.tile([C, NCOR], f32,
                                                   tag="corrs", name="corrs")
                                nc.scalar.activation(
                                    out=corrs[:], in_=nm[:, 0:NCOR],
                                    func=Act.Exp, bias=ngm[:], scale=-1.0,
                                )
                                nc.vector.tensor_tensor(
                                    out=rsc[:, 0:NCOR], in0=rsc[:, 0:NCOR],
                                    in1=corrs[:], op=Alu.mult,
                                )
                                for h in range(NCOR):
                                    nc.vector.tensor_scalar(
                                        out=P_t[:, h * MCHUNK:
                                                (h + 1) * MCHUNK],
                                        in0=P_t[:, h * MCHUNK:
                                                (h + 1) * MCHUNK],
                                        scalar1=corrs[:, h:h + 1],
                                        scalar2=None, op0=Alu.mult,
                                    )
                            rs = stats.tile([C, 1], f32, tag="rs", name="rs")
                            nc.vector.tensor_reduce(
                                out=rs[:], in_=rsc[:],
                                axis=mybir.AxisListType.X, op=Alu.add,
                            )
                            iv = ivp.tile([C, 1], f32, tag="iv", name="iv")
                            nc.vector.reciprocal(iv[:], rs[:])
                            iv_tiles[nt] = iv
                        return emit

                    def mk_fill(r, t=t, state=state):
                        # transpose 8 m-blocks of THIS tile right after its
                        # softmax -> keeps PE warm inside the S window
                        def emit():
                            if t == 0:
                                ptc_tiles[g][r] = ptc_pool.tile(
                                    [C, TPG, 1024], f16, tag="pt",
                                    name="ptc")
                            ptc_r = ptc_tiles[g][r]
                            P_t = state["P_t"]
                            tp = psT.tile([C, 1024], f16, tag="t", name="tp")
                            for k in range(8):
                                i = 8 * r + k
                                nc.tensor.transpose(
                                    tp[:, k * 128:(k + 1) * 128],
                                    P_t[:, i * 128:(i + 1) * 128],
                                    ident[:],
                                )
                            d = 4 * r + t
                            if d % 2 == 0 or d == 5:
                                nc.vector.tensor_copy(ptc_r[:, t, :], tp[:])
                            else:
                                nc.scalar.activation(out=ptc_r[:, t, :],
                                                     in_=tp[:],
                                                     func=Act.Copy)
                        return emit

                    for h in range(MCH_CNT):
                        units.append(mk_chunk(h))
                    units.append(mk_stats())
                    for r in range(MBLK // 8):
                        units.append(mk_fill(r))
                return units

            def tpv_units_for_group(g):
                """Emission closures for group g's transpose/PV/epilogue."""
                units = []
                state = {}

                def mk_pv(r, state=state):
                    def emit():
                        if r == 0:
                            state["O_ps"] = psA.tile([C, GRP], f32, tag="a",
                                                     name="O_ps")
                            xpb_g = small.tile([C, TPG, 128], f32, tag="xpb",
                                               name="xpb_g")
                            state["xpb_g"] = xpb_g
                            nc.gpsimd.dma_start(
                                out=xpb_g[:],
                                in_=xpb_e[g * GRP:(g + 1) * GRP, :].rearrange(
                                    "(t p) c -> p t c", p=128),
                            )
                        O_ps = state["O_ps"]
                        ptc_r = ptc_tiles[g][r]
                        for k in range(8):
                            i = 8 * r + k
                            nc.tensor.matmul(
                                O_ps[:], lhsT=V_sb[:, i, :],
                                rhs=ptc_r[:, :, k * 128:(k + 1) * 128],
                                start=(i == 0), stop=(i == MBLK - 1),
                            )
                    return emit

                def mk_epi(state=state):
                    def emit():
                        O_ps = state["O_ps"]
                        O_sb = small.tile([C, GRP], f16, tag="O", name="O_sb")
                        nc.scalar.activation(out=O_sb[:], in_=O_ps[:],
                                             func=Act.Copy)
                        H_ps = psA.tile([C, GRP], f32, tag="a", name="H_ps")
                        nc.tensor.matmul(H_ps[:], lhsT=pw_sb[:], rhs=O_sb[:])
                        Hs = small.tile([C, GRP], f16, tag="Hs", name="Hs")
                        nc.scalar.activation(out=Hs[:], in_=H_ps[:],
                                             func=Act.Copy)
                        tph = psA.tile([C, GRP], f16, tag="a", name="tph")
                        for t in range(TPG):
                            nc.tensor.transpose(
                                tph[:, t * 128:(t + 1) * 128],
                                Hs[:, t * 128:(t + 1) * 128], ident[:],
                            )
                        xpb_g = state["xpb_g"]
                        out_g = small.tile([C, TPG, 128], f32, tag="og",
                                           name="out_g")
                        for t in range(TPG):
                            nt = g * TPG + t
                            nc.vector.scalar_tensor_tensor(
                                out=out_g[:, t, :],
                                in0=tph[:, t * 128:(t + 1) * 128],
                                scalar=iv_tiles[nt][:], in1=xpb_g[:, t, :],
                                op0=Alu.mult, op1=Alu.add,
                            )
                        nc.gpsimd.dma_start(
                            out=out_e[g * GRP:(g + 1) * GRP, :].rearrange(
                                "(t p) c -> p t c", p=128),
                            in_=out_g[:],
                        )
                    return emit

                for r in range(MBLK // 8):
                    units.append(mk_pv(r))
                units.append(mk_epi())
                return units

            # ---- interleaved emission: S(g) with TPV(g-1) ----
            for g in range(NGRP + 1):
                su = s_units_for_group(g) if g < NGRP else []
                tu = tpv_units_for_group(g - 1) if g > 0 else v_units()
                ns, ntp = len(su), len(tu)
                total = max(ns, ntp)
                si = ti = 0
                for k in range(total):
                    while ti * total <= k * ntp and ti < ntp:
                        tu[ti]()
                        ti += 1
                    while si * total <= k * ns and si < ns:
                        su[si]()
                        si += 1
                while ti < ntp:
                    tu[ti]()
                    ti += 1
                while si < ns:
                    su[si]()
                    si += 1

    nc.compile()
    return nc


def _get_nc(n_tokens=N):
    if n_tokens not in _cache:
        _cache[n_tokens] = _build(n_tokens)
    return _cache[n_tokens]


def prep_inputs(x, qw, qb, kw, kb, vw, vb, proj_w, proj_b, n_tokens=N):
    """Host-side prep: shard over batch, fold scale/biases, transpose."""
    x = np.asarray(x, dtype=np.float32)
    b, c, h, w = x.shape
    scale = c ** (-0.5)
    qw_s = (np.asarray(qw, np.float32) * scale).astype(np.float16)
    kw16 = np.asarray(kw, np.float32).astype(np.float16)
    vw16 = np.asarray(vw, np.float32).astype(np.float16)
    pw16 = np.asarray(proj_w, np.float32).astype(np.float16)
    qb_s = (np.asarray(qb, np.float32) * scale).reshape(c, 1).astype(np.float32)
    pb2 = (np.asarray(vb, np.float32) @ np.asarray(proj_w, np.float32)
           + np.asarray(proj_b, np.float32)).astype(np.float32)

    in_maps = []
    for i in range(b):
        xc = x[i].reshape(c, h * w)[:, :n_tokens]
        xt = xc.T.copy()
        in_maps.append({
            "xh": np.ascontiguousarray(xc).astype(np.float16),
            "xpb": np.ascontiguousarray(xt + pb2[None, :]),
            "qw": qw_s, "kw": kw16, "vw": vw16, "pw": pw16,
            "qb": qb_s,
        })
    return in_maps


def kernel(x, qw, qb, kw, kb, vw, vb, proj_w, proj_b, _trace=False):
    from concourse.bass_utils import run_bass_kernel_spmd

    nc = _get_nc(N)
    in_maps = prep_inputs(x, qw, qb, kw, kb, vw, vb, proj_w, proj_b)
    res = run_bass_kernel_spmd(nc, in_maps, core_ids=list(range(B)),
                               trace=_trace)
    kernel.last_results = res
    out = np.stack([np.asarray(res.results[i]["out"]) for i in range(B)])
    return out.reshape(B, H, W, C).astype(np.float32)
